# revision 1
# baseline (speedup 1.0000x reference)
"""CTC loss (log_softmax over time + CTC forward DP) on 8 Trainium2 NeuronCores.

Two SPMD launches:

Phase 1 (time-sharded): core c owns time slice [c*T/8, (c+1)*T/8) of ALL
batches. It streams its [B, T/8, C] slab in [128, C] tiles (2 batches x 64
timesteps per tile), gathers each batch's 33 unique label columns (32 targets
+ blank) with baked copies — indices are identical across cores because every
core sees every batch — exponentiates, computes per-(column,batch) partial
sumexp over its time slice (PE matmul with a per-batch selector), and writes
the exp'd gather (33 x B x T/8, ~0.3 MB) back to HBM via the ACT DMA ring
(separate FIFO from the input loads).

Host: sums partial sumexps into q[s,b] = e^c0 / sumexp (the log_softmax-over-
time denominator, expanded to the 65 extended states), reassembles the
gathered data, and redistributes it for phase 2: pairs of cores split each
batch group's time range in halves; the backward half gets s- and t-reversed
data so both directions run the same program.

Phase 2 (batch+direction sharded): the CTC forward recursion in probability
space is the linear recurrence E' = (A @ E) * W_t (plus A2 @ (E*kmask) when
adjacent repeated labels exist), with the banded transition as resident PE
weights and the W multiply one DVE op from PSUM. W is built on-chip by a
constant 33->65 expansion matmul scaled by q. Every RENORM_EVERY steps E is
renormalized by its column sum (PE sum -> reciprocal -> PE rank-1 broadcast
-> multiply) to stay in f32 range; the log corrections accumulate off the
critical chain. Host combines the forward/backward halves per batch in f64.
"""

from contextlib import ExitStack

import numpy as np

import concourse.bacc as bacc
import concourse.tile as tile
from concourse import mybir
from concourse.bass_utils import run_bass_kernel_spmd

BLANK = 6624
N_CORES = 8
C0 = 5.64  # per-step rescale folded into W
RENORM_EVERY = 16

F32 = mybir.dt.float32
F32R = mybir.dt.float32r

LAST_RESULTS = None  # (phase1 BassKernelResults, phase2 BassKernelResults)
_P1_CACHE = {}
_P2_CACHE = {}

Exp = mybir.ActivationFunctionType.Exp
Ln = mybir.ActivationFunctionType.Ln


def _build_phase1(b_tot, t_slice, c_dim, u_dim, ucols):
    """Gather + exp + partial sumexp for all batches over this core's time
    slice. ucols: [b_tot, u_dim] baked gather columns (identical across
    cores). The per-tile column gather runs as one GPSIMD indirect_copy
    (per-16-partition-core index lists) instead of u_dim*bpt DVE copies,
    which kept DVE ~100% busy and serialized the whole phase."""
    bpt = min(max(1, 128 // t_slice), b_tot)
    assert bpt * t_slice <= 128, "time slice too large for one tile"
    assert b_tot % bpt == 0
    n_tiles = b_tot // bpt
    rows = bpt * t_slice
    islots = (u_dim + 15) // 16  # idx slots per partition (wrapped j%16, j//16)

    nc = bacc.Bacc("TRN2", num_devices=N_CORES)
    lp_t = nc.dram_tensor("lp", [b_tot, t_slice, c_dim], F32, kind="ExternalInput")
    ident_t = nc.dram_tensor("ident", [128, 128], F32, kind="ExternalInput")
    sel_t = nc.dram_tensor("sel", [128, bpt], F32, kind="ExternalInput")
    n_idx = islots * 16
    gidx_t = nc.dram_tensor(
        "gidx", [128, n_tiles * islots], mybir.dt.int16, kind="ExternalInput"
    )
    egb_t = nc.dram_tensor("egb", [u_dim, b_tot, t_slice], F32, kind="ExternalOutput")
    sq_t = nc.dram_tensor("sq", [u_dim, b_tot], F32, kind="ExternalOutput")

    with tile.TileContext(nc) as tc, ExitStack() as ctx:
        consts = ctx.enter_context(tc.tile_pool(name="consts", bufs=1))
        lp_pool = ctx.enter_context(tc.tile_pool(name="lp", bufs=3))
        eg_pool = ctx.enter_context(tc.tile_pool(name="eg", bufs=3))
        st_pool = ctx.enter_context(tc.tile_pool(name="st", bufs=3))
        sqs_pool = ctx.enter_context(tc.tile_pool(name="sqs", bufs=1))

        ident_sb = consts.tile([128, 128], F32, tag="ident")
        nc.sync.dma_start(out=ident_sb[:], in_=ident_t[:])
        sel = consts.tile([128, bpt], F32, tag="sel")
        nc.sync.dma_start(out=sel[:], in_=sel_t[:])
        gidx_sb = consts.tile([128, n_tiles * islots], mybir.dt.int16, tag="gidx")
        nc.sync.dma_start(out=gidx_sb[:], in_=gidx_t[:])

        with (
            tc.tile_pool(name="psq", bufs=1, space="PSUM") as psq_pool,
            tc.tile_pool(name="tp", bufs=3, space="PSUM") as tp_pool,
        ):
            psum_q = psq_pool.tile([u_dim, b_tot], F32, tag="psq")
            for k in range(n_tiles):
                b0 = k * bpt
                lpt = lp_pool.tile([rows, c_dim], F32, tag="lpt")
                nc.sync.dma_start(
                    out=lpt[:],
                    in_=lp_t[b0 : b0 + bpt, :, :].rearrange("b t c -> (b t) c"),
                )
                gath = eg_pool.tile([rows, n_idx], F32, tag="gath")
                nc.gpsimd.ap_gather(
                    out_ap=gath[:],
                    in_ap=lpt[:],
                    idxs_ap=gidx_sb[:, k * islots : (k + 1) * islots],
                    channels=rows,
                    num_elems=c_dim,
                    d=1,
                    num_idxs=n_idx,
                )
                eg = eg_pool.tile([rows, u_dim], F32, tag="eg")
                nc.scalar.activation(eg[:], gath[:, :u_dim], Exp)
                nc.tensor.matmul(
                    psum_q[:, b0 : b0 + bpt],
                    lhsT=eg[:],
                    rhs=sel[:],
                    start=True,
                    stop=True,
                )
                tp = tp_pool.tile([u_dim, rows], F32, tag="tp")
                nc.tensor.transpose(tp[:], eg[:], ident_sb[:])
                stg = st_pool.tile([u_dim, rows], F32, tag="stg")
                nc.vector.tensor_copy(stg[:], tp[:])
                # ACT's DMA ring: don't head-of-line block the lp loads on SP
                nc.scalar.dma_start(
                    out=egb_t[:, b0 : b0 + bpt, :].rearrange("s b t -> s (b t)"),
                    in_=stg[:],
                )
            sqs = sqs_pool.tile([u_dim, b_tot], F32, tag="sqs")
            nc.vector.tensor_copy(sqs[:], psum_q[:])
        nc.sync.dma_start(out=sq_t[:], in_=sqs[:])
    nc.finalize()
    return nc


def _build_phase2(bc, t_steps, s_dim, u_dim, use_a2):
    """The DP. All per-core differences are input data."""
    nc = bacc.Bacc("TRN2", num_devices=N_CORES)
    egb_t = nc.dram_tensor("egb", [u_dim, bc, t_steps], F32, kind="ExternalInput")
    q_t = nc.dram_tensor("q", [s_dim, bc], F32, kind="ExternalInput")
    expt_t = nc.dram_tensor("expt", [u_dim, s_dim], F32, kind="ExternalInput")
    a0t_t = nc.dram_tensor("a0t", [s_dim, s_dim], F32R, kind="ExternalInput")
    if use_a2:
        a2t_t = nc.dram_tensor("a2t", [s_dim, s_dim], F32R, kind="ExternalInput")
        km_t = nc.dram_tensor("kmask", [s_dim, bc], F32, kind="ExternalInput")
    init_t = nc.dram_tensor("init", [s_dim, bc], F32, kind="ExternalInput")
    ones_t = nc.dram_tensor("ones_s", [s_dim, 1], F32R, kind="ExternalInput")
    efin_t = nc.dram_tensor("efin", [s_dim, bc], F32, kind="ExternalOutput")
    lacc_t = nc.dram_tensor("lacc", [1, bc], F32, kind="ExternalOutput")

    with tile.TileContext(nc) as tc, ExitStack() as ctx:
        consts = ctx.enter_context(tc.tile_pool(name="consts", bufs=1))
        w_pool = ctx.enter_context(tc.tile_pool(name="w", bufs=1))
        e_pool = ctx.enter_context(tc.tile_pool(name="e", bufs=3))
        sm_pool = ctx.enter_context(tc.tile_pool(name="sm", bufs=2))
        out_pool = ctx.enter_context(tc.tile_pool(name="out", bufs=1))

        a0t_sb = consts.tile([s_dim, s_dim], F32R, tag="a0t")
        nc.sync.dma_start(out=a0t_sb[:], in_=a0t_t[:])
        if use_a2:
            a2t_sb = consts.tile([s_dim, s_dim], F32R, tag="a2t")
            nc.sync.dma_start(out=a2t_sb[:], in_=a2t_t[:])
            km_sb = consts.tile([s_dim, bc], F32, tag="km")
            nc.sync.dma_start(out=km_sb[:], in_=km_t[:])
            ek_pool = ctx.enter_context(tc.tile_pool(name="ek", bufs=2))
        init_sb = consts.tile([s_dim, bc], F32, tag="init")
        nc.sync.dma_start(out=init_sb[:], in_=init_t[:])
        q_sb = consts.tile([s_dim, bc], F32, tag="q")
        nc.sync.dma_start(out=q_sb[:], in_=q_t[:])
        expt_sb = consts.tile([u_dim, s_dim], F32, tag="expt")
        nc.sync.dma_start(out=expt_sb[:], in_=expt_t[:])
        ones_s = consts.tile([s_dim, 1], F32R, tag="ones_s")
        nc.sync.dma_start(out=ones_s[:], in_=ones_t[:])
        ones_row = consts.tile([1, s_dim], F32, tag="ones_row")
        nc.vector.memset(ones_row[:], 1.0)

        egb_sb = w_pool.tile([u_dim, bc, t_steps], F32, tag="egb")
        nc.sync.dma_start(out=egb_sb[:], in_=egb_t[:])
        warr = w_pool.tile([s_dim, bc, t_steps], F32, tag="warr")
        with tc.tile_pool(name="wx", bufs=2, space="PSUM") as wx_pool:
            for b in range(bc):
                wx = wx_pool.tile([s_dim, t_steps], F32, tag="wx")
                nc.tensor.matmul(
                    wx[:], lhsT=expt_sb[:], rhs=egb_sb[:, b, :], start=True, stop=True
                )
                nc.vector.tensor_scalar_mul(
                    warr[:, b, :], in0=wx[:], scalar1=q_sb[:, b : b + 1]
                )

        with (
            tc.tile_pool(name="p1", bufs=2, space="PSUM") as p_pool,
            tc.tile_pool(name="rs", bufs=2, space="PSUM") as rs_pool,
            tc.tile_pool(name="pb", bufs=2, space="PSUM") as pb_pool,
        ):
            E = e_pool.tile([s_dim, bc], F32R, tag="E")
            nc.vector.tensor_mul(E[:], init_sb[:], warr[:, :, 0])
            logacc = sm_pool.tile([1, bc], F32, tag="lg")
            nc.vector.memset(logacc[:], 0.0)

            for t in range(1, t_steps):
                p1 = p_pool.tile([s_dim, bc], F32, tag="p1")
                if use_a2:
                    ek = ek_pool.tile([s_dim, bc], F32R, tag="EK")
                    nc.vector.tensor_mul(ek[:], E[:], km_sb[:])
                    nc.tensor.matmul(
                        p1[:], lhsT=a0t_sb[:], rhs=E[:], start=True, stop=False
                    )
                    nc.tensor.matmul(
                        p1[:], lhsT=a2t_sb[:], rhs=ek[:], start=False, stop=True
                    )
                else:
                    nc.tensor.matmul(
                        p1[:], lhsT=a0t_sb[:], rhs=E[:], start=True, stop=True
                    )
                En = e_pool.tile([s_dim, bc], F32R, tag="E")
                nc.vector.tensor_mul(En[:], p1[:], warr[:, :, t])
                E = En

                if t % RENORM_EVERY == 0:
                    ps = rs_pool.tile([1, bc], F32, tag="ps")
                    nc.tensor.matmul(
                        ps[:], lhsT=ones_s[:], rhs=E[:], start=True, stop=True
                    )
                    rr = sm_pool.tile([1, bc], F32, tag="rr")
                    nc.vector.reciprocal(rr[:], ps[:])
                    # log correction runs off the serial chain
                    lnv = sm_pool.tile([1, bc], F32, tag="lnv")
                    nc.scalar.activation(lnv[:], ps[:], Ln)
                    lg2 = sm_pool.tile([1, bc], F32, tag="lg")
                    nc.vector.tensor_add(lg2[:], logacc[:], lnv[:])
                    logacc = lg2
                    pb = pb_pool.tile([s_dim, bc], F32, tag="pb")
                    nc.tensor.matmul(
                        pb[:], lhsT=ones_row[:], rhs=rr[:], start=True, stop=True
                    )
                    En2 = e_pool.tile([s_dim, bc], F32R, tag="E")
                    nc.vector.tensor_mul(En2[:], E[:], pb[:])
                    E = En2

            sv = out_pool.tile([s_dim, bc], F32, tag="sv")
            nc.vector.tensor_copy(sv[:], E[:])
            svl = out_pool.tile([1, bc], F32, tag="svl")
            nc.vector.tensor_copy(svl[:], logacc[:])
            nc.sync.dma_start(out=efin_t[:], in_=sv[:])
            nc.sync.dma_start(out=lacc_t[:], in_=svl[:])
    nc.finalize()
    return nc


def kernel(log_probs, targets, input_lengths, target_lengths):
    global LAST_RESULTS
    log_probs = np.asarray(log_probs, dtype=np.float32)
    tgt = np.asarray(targets).astype(np.int64)
    ilen = np.asarray(input_lengths).astype(np.int64)
    tlen = np.asarray(target_lengths).astype(np.int64)
    b_tot, t_len, c_dim = log_probs.shape
    l_max = tgt.shape[1]
    s_dim = 2 * l_max + 1
    u_dim = l_max + 1  # unique columns: labels + blank
    n_pairs = N_CORES // 2
    assert b_tot % n_pairs == 0
    bc = b_tot // n_pairs
    assert t_len % (2 * N_CORES) == 0
    t_slice = t_len // N_CORES
    t_half = t_len // 2
    assert (ilen == t_len).all(), "variable input_lengths not supported"

    ucols = np.concatenate(
        [tgt, np.full((b_tot, 1), BLANK, dtype=np.int64)], axis=1
    )  # [b, u]

    ext = np.full((b_tot, s_dim), BLANK, dtype=np.int64)
    ext[:, 1::2] = tgt
    ext_m2 = np.full_like(ext, BLANK)
    ext_m2[:, 2:] = ext[:, :-2]
    allow_skip = (ext != BLANK) & (ext != ext_m2)  # [b, s]

    # collisions among real labels force the two-matmul general path
    coll = False
    for b in range(b_tot):
        for s in range(3, min(2 * int(tlen[b]) + 1, s_dim), 2):
            if not allow_skip[b, s]:
                coll = True
    use_a2 = bool(coll)

    # s -> unique column map (same for every batch)
    smap = np.zeros(s_dim, dtype=np.int64)
    smap[0::2] = l_max
    smap[1::2] = np.arange(l_max)

    # ---- phase 1 ----
    key1 = (b_tot, t_slice, c_dim, u_dim, ucols.tobytes())
    if key1 not in _P1_CACHE:
        _P1_CACHE.clear()
        _P1_CACHE[key1] = _build_phase1(b_tot, t_slice, c_dim, u_dim, ucols)
    nc1 = _P1_CACHE[key1]

    ident = np.eye(128, dtype=np.float32)
    bpt = min(max(1, 128 // t_slice), b_tot)
    sel_np = np.zeros((128, bpt), dtype=np.float32)
    for h in range(bpt):
        sel_np[h * t_slice : (h + 1) * t_slice, h] = 1.0
    # per-tile gather indices, wrapped per 16-partition gpsimd core:
    # core index j lives at (partition j%16, slot j//16) of the core's rows
    assert t_slice % 16 == 0, "each gpsimd core must sit inside one batch row"
    n_tiles = b_tot // bpt
    islots = (u_dim + 15) // 16
    gidx_np = np.zeros((128, n_tiles * islots), dtype=np.int16)
    for k in range(n_tiles):
        for core in range(8):
            batch = k * bpt + (16 * core) // t_slice
            for s in range(islots):
                for pi in range(16):
                    j = s * 16 + pi
                    col = ucols[batch, j] if j < u_dim else ucols[batch, u_dim - 1]
                    gidx_np[16 * core + pi, k * islots + s] = col
    in_maps1 = []
    for c in range(N_CORES):
        sl = np.ascontiguousarray(log_probs[:, c * t_slice : (c + 1) * t_slice, :])
        in_maps1.append({"lp": sl, "ident": ident, "sel": sel_np, "gidx": gidx_np})
    res1 = run_bass_kernel_spmd(nc1, in_maps1, list(range(N_CORES)))

    sumexp = np.zeros((u_dim, b_tot), dtype=np.float64)
    egb_full = np.zeros((u_dim, b_tot, t_len), dtype=np.float32)
    for c in range(N_CORES):
        sumexp += res1.results[c]["sq"].astype(np.float64)
        egb_full[:, :, c * t_slice : (c + 1) * t_slice] = res1.results[c]["egb"]
    q65_full = (np.exp(C0) / sumexp[smap, :]).astype(np.float32)  # [s, b]

    # ---- phase 2 ----
    key2 = (bc, t_half, s_dim, u_dim, use_a2)
    if key2 not in _P2_CACHE:
        _P2_CACHE.clear()
        _P2_CACHE[key2] = _build_phase2(bc, t_half, s_dim, u_dim, use_a2)
    nc2 = _P2_CACHE[key2]

    # expansion matrices (fwd: s -> smap[s]; bwd: s-reversed)
    expt_f = np.zeros((u_dim, s_dim), dtype=np.float32)
    expt_f[smap, np.arange(s_dim)] = 1.0
    expt_b = np.zeros((u_dim, s_dim), dtype=np.float32)
    expt_b[smap[::-1], np.arange(s_dim)] = 1.0

    a0 = np.eye(s_dim, dtype=np.float64) + np.eye(s_dim, k=-1, dtype=np.float64)
    if use_a2:
        a0t_f = a0t_b = np.ascontiguousarray(a0.T).astype(np.float32)
        a2 = np.eye(s_dim, k=-2, dtype=np.float32)
        a2t = np.ascontiguousarray(a2.T)
    else:
        # fold the skip pattern (all odd states) into a single matrix
        acomb = a0.copy()
        for s in range(3, s_dim, 2):
            acomb[s, s - 2] = 1.0
        # backward: J A^T J has the same banded form with K~[s] = K[s_dim+1-s];
        # for the all-odd pattern K~ hits odd s too (s_dim odd => parity kept)
        a0t_f = np.ascontiguousarray(acomb.T).astype(np.float32)
        a0t_b = a0t_f  # symmetric pattern: K~[s]=K[s_dim+1-s], odd->odd
    in_maps2 = []
    for c in range(N_CORES):
        pair = c // 2
        fwd = c % 2 == 0
        bs = slice(pair * bc, (pair + 1) * bc)
        init = np.zeros((s_dim, bc), dtype=np.float32)
        km = np.zeros((s_dim, bc), dtype=np.float32)
        if fwd:
            egb = np.ascontiguousarray(egb_full[:, bs, :t_half])
            qv = np.ascontiguousarray(q65_full[:, bs])
            expt = expt_f
            a0t_c = a0t_f
            km[: s_dim - 2, :] = allow_skip[bs, 2:].T
            init[0, :] = 1.0
            init[1, :] = 1.0
        else:
            egb = np.ascontiguousarray(egb_full[:, bs, : t_half - 1 : -1])
            qv = np.ascontiguousarray(q65_full[::-1, bs])
            expt = expt_b
            a0t_c = a0t_b
            for bi, bg in enumerate(range(pair * bc, (pair + 1) * bc)):
                for u in range(s_dim - 2):
                    km[u, bi] = allow_skip[bg, s_dim - 1 - u]
                lb = int(tlen[bg])
                i1 = 2 * lb
                i2 = max(2 * lb - 1, 0)
                init[s_dim - 1 - i1, bi] = 1.0
                init[s_dim - 1 - i2, bi] += 1.0
        m = {"egb": egb, "q": qv, "expt": expt, "a0t": a0t_c, "init": init,
             "ones_s": np.ones((s_dim, 1), dtype=np.float32)}
        if use_a2:
            m["a2t"] = a2t
            m["kmask"] = km
        in_maps2.append(m)
    res2 = run_bass_kernel_spmd(nc2, in_maps2, list(range(N_CORES)))
    LAST_RESULTS = (res1, res2)

    # ---- host combine (float64) ----
    losses = np.zeros(b_tot, dtype=np.float64)
    for pair in range(n_pairs):
        cf, cb = 2 * pair, 2 * pair + 1
        ef = res2.results[cf]["efin"].astype(np.float64)
        lf = res2.results[cf]["lacc"].astype(np.float64)[0]
        eb = res2.results[cb]["efin"].astype(np.float64)
        lb_ = res2.results[cb]["lacc"].astype(np.float64)[0]
        for bi in range(bc):
            bg = pair * bc + bi
            y = eb[::-1, bi]
            ab = np.eye(s_dim) + np.eye(s_dim, k=-1)
            for s in range(2, s_dim):
                if allow_skip[bg, s]:
                    ab[s, s - 2] = 1.0
            u = ab.T @ y
            val = float(u @ ef[:, bi])
            lam = lf[bi] + lb_[bi]
            if not np.isfinite(val) or val <= 0.0:
                loss = np.inf
            else:
                loss = -(np.log(val) - t_len * C0 + lam)
            if loss > 1e20:
                loss = 0.0  # zero_infinity
            losses[bg] = loss / max(int(tlen[bg]), 1)
    return np.float32(losses.mean())



# revision 2
# speedup vs baseline: 1.1718x; 1.1718x over previous
"""CTC loss (log_softmax over time + CTC forward DP) on 8 Trainium2 NeuronCores.

Phase 1 (time-sharded, unchanged): core c owns time slice [c*T/8, (c+1)*T/8)
of ALL batches; streams its [B, T/8, C] slab, gathers each batch's 33 unique
label columns (32 targets + blank) on GPSIMD, exponentiates, computes partial
sumexp over its slice, writes the exp'd gather back to HBM.

Host: sums partial sumexps into the log_softmax-over-time denominator,
builds per-row W = q * exp(gathered) arrays for phase 2.

Phase 2 (scan-based): the CTC DP is reordered: instead of 2*T serial steps
over t, it runs S=65 steps over the extended-label axis s. For a fixed s,
alpha[t,s] = (alpha[t-1,s] + tmp[t]) * W[t,s] with
tmp = alpha[t-1,s-1] + K[s]*alpha[t-1,s-2] is a first-order affine
recurrence along t -- exactly DVE's tensor_tensor_scan (state =
(data0 + state) * data1, one independent recurrence per partition).
Rows = (batch, direction): each of the 8 cores runs 4 batches x {fwd, bwd
(s- and t-reversed data)} as 8 partition rows; per s: one scalar_tensor_tensor
(odd s only) + one scan over the 256-step half. fp32 range is handled by a
per-step constant e^{-C0} folded into W plus one max-rescale of the
t=127 boundary column (scales output consistently; logged and corrected on
the host). Host combines fwd/bwd halves per batch in f64.
"""

from contextlib import ExitStack

import numpy as np

import concourse.bacc as bacc
import concourse.tile as tile
from concourse import mybir
from concourse.bass_utils import run_bass_kernel_spmd

BLANK = 6624
N_CORES = 8
C0 = 6.1  # per-step rescale folded into W (keeps fp32 range in the scan)
CHUNK = 128  # scan t-chunk between boundary rescales

F32 = mybir.dt.float32

LAST_RESULTS = None  # (phase1 BassKernelResults, phase2 BassKernelResults)
_P1_CACHE = {}
_P2_CACHE = {}

Exp = mybir.ActivationFunctionType.Exp
ADD = mybir.AluOpType.add
MULT = mybir.AluOpType.mult


def _build_phase1(b_tot, t_slice, c_dim, u_dim, ucols):
    """Gather + exp + partial sumexp for all batches over this core's time
    slice. ucols: [b_tot, u_dim] baked gather columns (identical across
    cores). The per-tile column gather runs as one GPSIMD indirect_copy
    (per-16-partition-core index lists) instead of u_dim*bpt DVE copies,
    which kept DVE ~100% busy and serialized the whole phase."""
    bpt = min(max(1, 128 // t_slice), b_tot)
    assert bpt * t_slice <= 128, "time slice too large for one tile"
    assert b_tot % bpt == 0
    n_tiles = b_tot // bpt
    rows = bpt * t_slice
    islots = (u_dim + 15) // 16  # idx slots per partition (wrapped j%16, j//16)

    nc = bacc.Bacc("TRN2", num_devices=N_CORES)
    lp_t = nc.dram_tensor("lp", [b_tot, t_slice, c_dim], F32, kind="ExternalInput")
    ident_t = nc.dram_tensor("ident", [128, 128], F32, kind="ExternalInput")
    sel_t = nc.dram_tensor("sel", [128, bpt], F32, kind="ExternalInput")
    n_idx = islots * 16
    gidx_t = nc.dram_tensor(
        "gidx", [128, n_tiles * islots], mybir.dt.int16, kind="ExternalInput"
    )
    egb_t = nc.dram_tensor("egb", [u_dim, b_tot, t_slice], F32, kind="ExternalOutput")
    sq_t = nc.dram_tensor("sq", [u_dim, b_tot], F32, kind="ExternalOutput")

    with tile.TileContext(nc) as tc, ExitStack() as ctx:
        consts = ctx.enter_context(tc.tile_pool(name="consts", bufs=1))
        lp_pool = ctx.enter_context(tc.tile_pool(name="lp", bufs=3))
        eg_pool = ctx.enter_context(tc.tile_pool(name="eg", bufs=3))
        st_pool = ctx.enter_context(tc.tile_pool(name="st", bufs=3))
        sqs_pool = ctx.enter_context(tc.tile_pool(name="sqs", bufs=1))

        ident_sb = consts.tile([128, 128], F32, tag="ident")
        nc.sync.dma_start(out=ident_sb[:], in_=ident_t[:])
        sel = consts.tile([128, bpt], F32, tag="sel")
        nc.sync.dma_start(out=sel[:], in_=sel_t[:])
        gidx_sb = consts.tile([128, n_tiles * islots], mybir.dt.int16, tag="gidx")
        nc.sync.dma_start(out=gidx_sb[:], in_=gidx_t[:])

        with (
            tc.tile_pool(name="psq", bufs=1, space="PSUM") as psq_pool,
            tc.tile_pool(name="tp", bufs=3, space="PSUM") as tp_pool,
        ):
            psum_q = psq_pool.tile([u_dim, b_tot], F32, tag="psq")
            for k in range(n_tiles):
                b0 = k * bpt
                lpt = lp_pool.tile([rows, c_dim], F32, tag="lpt")
                nc.sync.dma_start(
                    out=lpt[:],
                    in_=lp_t[b0 : b0 + bpt, :, :].rearrange("b t c -> (b t) c"),
                )
                gath = eg_pool.tile([rows, n_idx], F32, tag="gath")
                nc.gpsimd.ap_gather(
                    out_ap=gath[:],
                    in_ap=lpt[:],
                    idxs_ap=gidx_sb[:, k * islots : (k + 1) * islots],
                    channels=rows,
                    num_elems=c_dim,
                    d=1,
                    num_idxs=n_idx,
                )
                eg = eg_pool.tile([rows, u_dim], F32, tag="eg")
                nc.scalar.activation(eg[:], gath[:, :u_dim], Exp)
                nc.tensor.matmul(
                    psum_q[:, b0 : b0 + bpt],
                    lhsT=eg[:],
                    rhs=sel[:],
                    start=True,
                    stop=True,
                )
                tp = tp_pool.tile([u_dim, rows], F32, tag="tp")
                nc.tensor.transpose(tp[:], eg[:], ident_sb[:])
                stg = st_pool.tile([u_dim, rows], F32, tag="stg")
                nc.vector.tensor_copy(stg[:], tp[:])
                # ACT's DMA ring: don't head-of-line block the lp loads on SP
                nc.scalar.dma_start(
                    out=egb_t[:, b0 : b0 + bpt, :].rearrange("s b t -> s (b t)"),
                    in_=stg[:],
                )
            sqs = sqs_pool.tile([u_dim, b_tot], F32, tag="sqs")
            nc.vector.tensor_copy(sqs[:], psum_q[:])
        nc.sync.dma_start(out=sq_t[:], in_=sqs[:])
    nc.finalize()
    return nc


def _chunk_bounds(t_half):
    """Chunk [1, t_half) into CHUNK-sized t-ranges; t=0 is handled apart."""
    bounds = []
    t0 = 1
    while t0 < t_half:
        t1 = min(t0 - 1 + CHUNK, t_half) if t0 == 1 else min(t0 + CHUNK, t_half)
        bounds.append((t0, t1))
        t0 = t1
    return bounds


def _build_phase2(rows, t_half, s_dim):
    """The scan-based DP. All per-core differences are input data.

    alpha buffer columns: col c holds alpha[t=c-1]; col 0 = the t=-1 init
    vector's image is not stored (t=0 handled by one elementwise multiply
    into col 1)."""
    chunks = _chunk_bounds(t_half)
    n_bnd = len(chunks) - 1

    nc = bacc.Bacc("TRN2", num_devices=N_CORES)
    w_t = nc.dram_tensor("w", [rows, s_dim, t_half], F32, kind="ExternalInput")
    k_t = nc.dram_tensor("k", [rows, s_dim], F32, kind="ExternalInput")
    init_t = nc.dram_tensor("init", [rows, s_dim], F32, kind="ExternalInput")
    aout_t = nc.dram_tensor("aout", [rows, s_dim], F32, kind="ExternalOutput")
    scl_t = nc.dram_tensor("scl", [rows, max(n_bnd, 1)], F32, kind="ExternalOutput")

    with tile.TileContext(nc) as tc, ExitStack() as ctx:
        pool = ctx.enter_context(tc.tile_pool(name="main", bufs=1))
        sm_pool = ctx.enter_context(tc.tile_pool(name="sm", bufs=2))

        wbuf = pool.tile([rows, s_dim, t_half], F32, tag="w")
        nc.sync.dma_start(out=wbuf[:], in_=w_t[:])
        kbuf = pool.tile([rows, s_dim], F32, tag="k")
        nc.sync.dma_start(out=kbuf[:], in_=k_t[:])
        ibuf = pool.tile([rows, s_dim], F32, tag="init")
        nc.sync.dma_start(out=ibuf[:], in_=init_t[:])

        abuf = pool.tile([rows, s_dim, t_half + 1], F32, tag="alpha")
        zrow = pool.tile([rows, CHUNK], F32, tag="zrow")
        nc.vector.memset(zrow[:], 0.0)
        sclbuf = pool.tile([rows, max(n_bnd, 1)], F32, tag="scl")
        nc.vector.memset(sclbuf[:], 1.0)

        # t=0: alpha[0, s] = init[s] * W[0, s]  (writes col 1, all s at once)
        nc.vector.tensor_mul(abuf[:, :, 1], ibuf[:], wbuf[:, :, 0])

        for ci, (t0, t1) in enumerate(chunks):
            n = t1 - t0
            if ci > 0:
                # rescale boundary column (alpha[t0-1] = col t0) by 1/max
                cm = sm_pool.tile([rows, 1], F32, tag="cm")
                nc.vector.tensor_reduce(
                    cm[:], abuf[:, :, t0], axis=mybir.AxisListType.X,
                    op=mybir.AluOpType.max,
                )
                nc.vector.tensor_copy(sclbuf[:, ci - 1 : ci], cm[:])
                rc = sm_pool.tile([rows, 1], F32, tag="rc")
                nc.vector.reciprocal(rc[:], cm[:])
                nc.vector.tensor_scalar_mul(
                    abuf[:, :, t0], in0=abuf[:, :, t0], scalar1=rc[:]
                )
            for s in range(s_dim):
                if s == 0:
                    d0 = zrow[:, :n]
                elif s == 1 or s % 2 == 0:
                    # no skip path into s: tmp = alpha[t-1, s-1] directly
                    d0 = abuf[:, s - 1, t0 : t0 + n]
                else:
                    tmp = sm_pool.tile([rows, CHUNK], F32, tag="tmp")
                    nc.vector.scalar_tensor_tensor(
                        out=tmp[:, :n],
                        in0=abuf[:, s - 2, t0 : t0 + n],
                        scalar=kbuf[:, s : s + 1],
                        in1=abuf[:, s - 1, t0 : t0 + n],
                        op0=MULT,
                        op1=ADD,
                    )
                    d0 = tmp[:, :n]
                nc.vector.tensor_tensor_scan(
                    out=abuf[:, s, t0 + 1 : t1 + 1],
                    data0=d0,
                    data1=wbuf[:, s, t0:t1],
                    initial=abuf[:, s, t0 : t0 + 1],
                    op0=ADD,
                    op1=MULT,
                )

        aout = pool.tile([rows, s_dim], F32, tag="aout")
        nc.vector.tensor_copy(aout[:], abuf[:, :, t_half])
        nc.sync.dma_start(out=aout_t[:], in_=aout[:])
        nc.sync.dma_start(out=scl_t[:], in_=sclbuf[:])
    nc.finalize()
    return nc


def kernel(log_probs, targets, input_lengths, target_lengths):
    global LAST_RESULTS
    log_probs = np.asarray(log_probs, dtype=np.float32)
    tgt = np.asarray(targets).astype(np.int64)
    ilen = np.asarray(input_lengths).astype(np.int64)
    tlen = np.asarray(target_lengths).astype(np.int64)
    b_tot, t_len, c_dim = log_probs.shape
    l_max = tgt.shape[1]
    s_dim = 2 * l_max + 1
    u_dim = l_max + 1  # unique columns: labels + blank
    assert b_tot % N_CORES == 0
    bc = b_tot // N_CORES  # batches per core in phase 2
    rows = 2 * bc  # fwd + bwd rows per core
    assert t_len % (2 * N_CORES) == 0
    t_slice = t_len // N_CORES
    t_half = t_len // 2
    assert (ilen == t_len).all(), "variable input_lengths not supported"

    ucols = np.concatenate(
        [tgt, np.full((b_tot, 1), BLANK, dtype=np.int64)], axis=1
    )  # [b, u]

    ext = np.full((b_tot, s_dim), BLANK, dtype=np.int64)
    ext[:, 1::2] = tgt
    ext_m2 = np.full_like(ext, BLANK)
    ext_m2[:, 2:] = ext[:, :-2]
    allow_skip = (ext != BLANK) & (ext != ext_m2)  # [b, s]

    # s -> unique column map (same for every batch)
    smap = np.zeros(s_dim, dtype=np.int64)
    smap[0::2] = l_max
    smap[1::2] = np.arange(l_max)

    # ---- phase 1 ----
    key1 = (b_tot, t_slice, c_dim, u_dim, ucols.tobytes())
    if key1 not in _P1_CACHE:
        _P1_CACHE.clear()
        _P1_CACHE[key1] = _build_phase1(b_tot, t_slice, c_dim, u_dim, ucols)
    nc1 = _P1_CACHE[key1]

    ident = np.eye(128, dtype=np.float32)
    bpt = min(max(1, 128 // t_slice), b_tot)
    sel_np = np.zeros((128, bpt), dtype=np.float32)
    for h in range(bpt):
        sel_np[h * t_slice : (h + 1) * t_slice, h] = 1.0
    # per-tile gather indices, wrapped per 16-partition gpsimd core:
    # core index j lives at (partition j%16, slot j//16) of the core's rows
    assert t_slice % 16 == 0, "each gpsimd core must sit inside one batch row"
    n_tiles = b_tot // bpt
    islots = (u_dim + 15) // 16
    gidx_np = np.zeros((128, n_tiles * islots), dtype=np.int16)
    for k in range(n_tiles):
        for core in range(8):
            batch = k * bpt + (16 * core) // t_slice
            for s in range(islots):
                for pi in range(16):
                    j = s * 16 + pi
                    col = ucols[batch, j] if j < u_dim else ucols[batch, u_dim - 1]
                    gidx_np[16 * core + pi, k * islots + s] = col
    in_maps1 = []
    for c in range(N_CORES):
        sl = np.ascontiguousarray(log_probs[:, c * t_slice : (c + 1) * t_slice, :])
        in_maps1.append({"lp": sl, "ident": ident, "sel": sel_np, "gidx": gidx_np})
    res1 = run_bass_kernel_spmd(nc1, in_maps1, list(range(N_CORES)))

    sumexp = np.zeros((u_dim, b_tot), dtype=np.float64)
    egb_full = np.zeros((u_dim, b_tot, t_len), dtype=np.float32)
    for c in range(N_CORES):
        sumexp += res1.results[c]["sq"].astype(np.float64)
        egb_full[:, :, c * t_slice : (c + 1) * t_slice] = res1.results[c]["egb"]
    q_full = (np.exp(C0) / sumexp[smap, :]).astype(np.float32)  # [s, b]

    # ---- phase 2 (scan DP) ----
    key2 = (rows, t_half, s_dim)
    if key2 not in _P2_CACHE:
        _P2_CACHE.clear()
        _P2_CACHE[key2] = _build_phase2(rows, t_half, s_dim)
    nc2 = _P2_CACHE[key2]

    smap_r = smap[::-1]
    in_maps2 = []
    for c in range(N_CORES):
        w_np = np.zeros((rows, s_dim, t_half), dtype=np.float32)
        k_np = np.zeros((rows, s_dim), dtype=np.float32)
        init_np = np.zeros((rows, s_dim), dtype=np.float32)
        for j in range(bc):
            b = c * bc + j
            eg_b = egb_full[:, b, :]  # [u, T]
            # fwd row j
            w_np[j] = q_full[:, b][:, None] * eg_b[smap, :t_half]
            k_np[j, 2:] = allow_skip[b, 2:]
            init_np[j, 0] = 1.0
            # bwd row bc + j: t reversed (T-1 .. t_half), s reversed
            w_np[bc + j] = q_full[::-1, b][:, None] * eg_b[smap_r, : t_half - 1 : -1]
            for sp in range(2, s_dim):
                k_np[bc + j, sp] = allow_skip[b, s_dim - 1 - (sp - 2)]
            lb = int(tlen[b])
            i1 = 2 * lb
            i2 = max(2 * lb - 1, 0)
            init_np[bc + j, s_dim - 1 - i1] = 1.0
            init_np[bc + j, s_dim - 1 - i2] += 1.0
        in_maps2.append({"w": w_np, "k": k_np, "init": init_np})
    res2 = run_bass_kernel_spmd(nc2, in_maps2, list(range(N_CORES)))
    LAST_RESULTS = (res1, res2)

    # ---- host combine (float64) ----
    losses = np.zeros(b_tot, dtype=np.float64)
    for c in range(N_CORES):
        aout = res2.results[c]["aout"].astype(np.float64)  # [rows, s]
        scl = res2.results[c]["scl"].astype(np.float64)  # [rows, n_bnd]
        lam_rows = np.log(np.maximum(scl, 1e-300)).sum(axis=1)
        for j in range(bc):
            b = c * bc + j
            ef = aout[j]
            y = aout[bc + j][::-1]
            abm = np.eye(s_dim) + np.eye(s_dim, k=-1)
            for s in range(2, s_dim):
                if allow_skip[b, s]:
                    abm[s, s - 2] = 1.0
            u = abm.T @ y
            val = float(u @ ef)
            lam = lam_rows[j] + lam_rows[bc + j]
            if not np.isfinite(val) or val <= 0.0:
                loss = np.inf
            else:
                loss = -(np.log(val) - t_len * C0 + lam)
            if loss > 1e20:
                loss = 0.0  # zero_infinity
            losses[b] = loss / max(int(tlen[b]), 1)
    return np.float32(losses.mean())
